# revision 19
# baseline (speedup 1.0000x reference)
"""Trainium2 Bass kernel for nn_Attention_9431748182617.

Quirky attention: scores z[b,k,q] = (q_h . k_h) / sqrt(D), softmax over the
QUERY axis (per key row), out[q] = sum_k A[k,q] * v[k], then output projection.

Sharding (8 NeuronCores):
  - tensor-parallel over heads: 16 heads -> 2 heads per core.
    Each core owns rows [128c, 128c+128) of Wq/Wk/Wv (its 2 heads) and
    computes q/k/v + attention for those heads over the full batch.
  - z^T (local 128 rows of L, all of B*S) is AllGather'd per batch.
  - output projection sharded by output feature D: core c computes
    out^T rows [128c, 128c+128) using Wo^T[:, 128c:128c+128] for ALL s.
  - host concatenates the 8 out^T blocks and transposes.

Matmuls in bf16 (fp32 PSUM accumulation), except the Q/K projections which
run fp8e4m3 DoubleRow (2 contraction rows/cell, half the stream cycles —
their quantization only perturbs softmax scores, ~1.1e-2 total rel err).
V/Wo stay bf16 (their element error reaches the output directly). exp on
ScalarE in fp32 with fused free-axis accumulation for softmax denominators;
1/denom is folded into V rows (per-partition scalar) so no full-size
normalization pass. V^T->V transposes ride the DMA xbar (PE is
power-throttled; every PE cycle counts).
"""

import os

import numpy as np
import ml_dtypes

import concourse.bass as bass
import concourse.mybir as mybir
import concourse.tile as tile
from concourse.bass_utils import run_bass_kernel_spmd
from concourse.masks import make_identity

B, S, D = 4, 2048, 1024
L, H = 1024, 16
DH = L // H               # 64
NCORES = 8
LPC = L // NCORES         # 128 l-rows (= 2 heads) per core
DPC = D // NCORES         # 128 out-feature rows per core
SCALE = 1.0 / (D ** 0.5)
KC = S // 128             # 16 key chunks of 128
BF16 = mybir.dt.bfloat16
F32 = mybir.dt.float32
F8 = mybir.dt.float8e4
EXP = mybir.ActivationFunctionType.Exp

LAST_EXEC_NS = None


def _body(tc, xT, x8, wq8, wk8, wvT, woT, outT, zloc, zfull):
    nc = tc.nc
    from contextlib import ExitStack

    with ExitStack() as ctx:
        const = ctx.enter_context(tc.tile_pool(name="const", bufs=1))
        xpool = ctx.enter_context(tc.tile_pool(name="xpool", bufs=1))
        qk = ctx.enter_context(tc.tile_pool(name="qk", bufs=2))
        vtpool = ctx.enter_context(tc.tile_pool(name="vtpool", bufs=1))
        vpool = ctx.enter_context(tc.tile_pool(name="vpool", bufs=2))
        apool = ctx.enter_context(tc.tile_pool(name="apool", bufs=7))
        small = ctx.enter_context(tc.tile_pool(name="small", bufs=8))
        ztp = ctx.enter_context(tc.tile_pool(name="ztp", bufs=2))
        zslab = ctx.enter_context(tc.tile_pool(name="zslab", bufs=2))
        osb_p = ctx.enter_context(tc.tile_pool(name="osb_p", bufs=2))
        # all 8 PSUM banks in one 4-deep [128,1024] pool: scores, AV
        # partials, projections, out-projection all cycle through it
        ps = ctx.enter_context(tc.tile_pool(name="ps", bufs=1, space="PSUM"))

        # ---- constants: weights ----
        # Q/K weights in fp8 (DoubleRow: 2 contraction rows per cell);
        # V/Wo stay bf16 — their element error reaches the output directly.
        wq_sb = const.tile([128, 4, 2, 128], F8, name="wq_sb")
        wk_sb = const.tile([128, 4, 2, 128], F8, name="wk_sb")
        nc.sync.dma_start(wq_sb, wq8)
        nc.sync.dma_start(wk_sb, wk8)
        wv_sb = const.tile([128, 8, 128], BF16, name="wv_sb")
        wo_sb = const.tile([128, 8, 128], BF16, name="wo_sb")
        for dc in range(8):
            nc.sync.dma_start(wv_sb[:, dc, :], wvT[dc * 128:(dc + 1) * 128, :])
            nc.sync.dma_start(wo_sb[:, dc, :], woT[dc * 128:(dc + 1) * 128, :])
        # fire the exp table load (~2.7us) under the startup DMAs instead
        # of paying it at the first real softmax activation
        warm_in = const.tile([128, 1], F32, name="warm_in")
        warm_out = const.tile([128, 1], F32, name="warm_out")
        nc.vector.memset(warm_in, 0.0)
        nc.scalar.activation(warm_out, warm_in, EXP)

        def load_x(b):
            # x8 rides the SWDGE queue (dedicated, drains in issue order so
            # the Q/K projections unblock first); the bigger xT goes to the
            # HWDGE rings in parallel.
            x8_c = []
            for j in range(4):
                xc = xpool.tile([128, 2, S], F8, name=f"x8c{j}", tag=f"x8{j}")
                nc.gpsimd.dma_start(xc, x8[b, j])
                x8_c.append(xc)
            x_c = []
            for dc in range(8):
                xc = xpool.tile([128, S], BF16, name=f"xc{dc}", tag=f"x{dc}")
                nc.sync.dma_start(xc, xT[b, dc * 128:(dc + 1) * 128, :])
                x_c.append(xc)
            return x_c + x8_c

        def proj_w_half(w_sb, dest, x_c, half):
            """One s-half of the V^T projection — bf16, weight-stationary."""
            pw = ps.tile([128, 1024], F32, name="pw", tag="work", bufs=2)
            for dc in range(8):
                for q in range(2):
                    sc = half * 2 + q
                    nc.tensor.matmul(
                        pw[:, q * 512:(q + 1) * 512],
                        lhsT=w_sb[:, dc, :],
                        rhs=x_c[dc][:, sc * 512:(sc + 1) * 512],
                        start=(dc == 0),
                        stop=(dc == 7),
                    )
            nc.vector.tensor_copy(dest[:, half * 1024:(half + 1) * 1024], pw)

        def proj_w8(w_sb, nm, x_c):
            """Q/K projection in fp8 DoubleRow."""
            dest = qk.tile([128, S], BF16, name=nm, tag=nm)
            for half in range(2):
                pw = ps.tile([128, 1024], F32, name="pw8", tag="work", bufs=2)
                for j in range(4):
                    for q in range(2):
                        sc = half * 2 + q
                        nc.tensor.matmul(
                            pw[:, q * 512:(q + 1) * 512],
                            lhsT=w_sb[:, j, :, :],
                            rhs=x_c[8 + j][:, :, sc * 512:(sc + 1) * 512],
                            start=(j == 0),
                            stop=(j == 3),
                            perf_mode=mybir.MatmulPerfMode.DoubleRow,
                        )
                nc.vector.tensor_copy(dest[:, half * 1024:(half + 1) * 1024],
                                      pw)
            return dest

        def transpose_v(vt, v_sb):
            # transpose VT [dh2, s] -> V [s, dh2] in 128-chunks on the DMA
            # xbar: PE is power-throttled, so keep transposes off it.
            # v_sb was allocated earlier (scores_exp references the handle
            # before this data lands; only the DVE vs-scaling waits on it).
            for c in range(KC):
                nc.sync.dma_start_transpose(
                    v_sb[:, c, :], vt[:, c * 128:(c + 1) * 128])

        def scores_exp(kc, qt, kt, v_sb):
            """Scores + exp + denominators + scaled V for key-chunk kc.
            Scores PSUM tiles use their own 2-deep rotation ("psc" tag) so
            the exp pacing never waits on AV folds or projection copies."""
            a_ts = [
                apool.tile([128, S], BF16, name=f"a{h}", tag=f"a{h}")
                for h in range(2)
            ]
            accs = [[], []]
            for half in range(2):
                tiles = [
                    ps.tile([128, 1024], F32, name=f"psc{h}", tag="psc",
                            bufs=2)
                    for h in range(2)
                ]
                for qq in range(2):
                    q0 = half * 1024 + qq * 512
                    for h in range(2):
                        hp = h * 64
                        nc.tensor.matmul(
                            tiles[h][:, qq * 512:(qq + 1) * 512],
                            lhsT=kt[hp:hp + 64, kc * 128:(kc + 1) * 128],
                            rhs=qt[hp:hp + 64, q0:q0 + 512],
                            start=True,
                            stop=True,
                        )
                for h in range(2):
                    acc = small.tile([128, 1], F32, name="acc", tag="acc")
                    nc.scalar.activation(
                        a_ts[h][:, half * 1024:(half + 1) * 1024],
                        tiles[h],
                        EXP,
                        scale=float(SCALE),
                        accum_out=acc,
                    )
                    accs[h].append(acc)
            res = []
            for h in range(2):
                den = small.tile([128, 1], F32, name="den", tag="den")
                nc.vector.tensor_add(den, accs[h][0], accs[h][1])
                rec = small.tile([128, 1], F32, name="rec", tag="rec")
                nc.vector.reciprocal(rec, den)
                vs = small.tile([128, DH], BF16, name="vs", tag=f"vs{h}")
                nc.vector.tensor_scalar_mul(
                    vs, v_sb[:, kc, h * 64:h * 64 + 64], rec)
                res.append((a_ts[h], vs))
            return res

        def av_pair(units, zac, first):
            """AV for two kc units: dense 16-matmul burst into two PSUM
            tiles (accumulating over the 2 kc), then fold into the SBUF
            f32 accumulator on DVE."""
            zps = [
                ps.tile([128, 1024], F32, name=f"zp{q2}", tag="work", bufs=2)
                for q2 in range(2)
            ]
            last = len(units) - 1
            for j, (kc, pair) in enumerate(units):
                for qc in range(4):
                    for h in range(2):
                        a_t, vs = pair[h]
                        hp = h * 64
                        nc.tensor.matmul(
                            zps[qc // 2][hp:hp + 64,
                                         (qc % 2) * 512:(qc % 2 + 1) * 512],
                            lhsT=vs,
                            rhs=a_t[:, qc * 512:(qc + 1) * 512],
                            start=(j == 0),
                            stop=(j == last),
                            skip_group_check=True,
                        )
            for q2 in range(2):
                sl = zac[:, q2 * 1024:(q2 + 1) * 1024]
                if first:
                    nc.vector.tensor_copy(sl, zps[q2])
                else:
                    nc.vector.tensor_add(sl, zps[q2], sl)

        def flush_half(b, zac, half):
            # zloc store needs the f32 -> bf16 cast, which only the SWDGE
            # path provides; the AllGather rides the same queue
            nc.gpsimd.dma_start(
                zloc[b, half], zac[:, half * 1024:(half + 1) * 1024])
            nc.gpsimd.collective_compute(
                "AllGather",
                mybir.AluOpType.bypass,
                replica_groups=[list(range(NCORES))],
                ins=[zloc[b, half].opt()],
                outs=[zfull[2 * b + half][:, :].opt()],
            )

        def outproj_load_half(b, half):
            """Prefetch the gathered z^T slabs for one s-half on HWDGE."""
            zf_c = []
            for j in range(4):
                zf = zslab.tile([128, 2, S // 2], BF16, name=f"zf{j}",
                                tag=f"zf{j}")
                nc.sync.dma_start(
                    zf,
                    zfull[2 * b + half][j * 256:(j + 1) * 256, :]
                    .rearrange("(c p) s -> p c s", p=128),
                )
                zf_c.append(zf)
            return zf_c

        def outproj_half(b, half, zf_c):
            po = ps.tile([128, 1024], F32, name="po", tag="work", bufs=2)
            for lc in range(8):
                for sc in range(2):
                    nc.tensor.matmul(
                        po[:, sc * 512:(sc + 1) * 512],
                        lhsT=wo_sb[:, lc, :],
                        rhs=zf_c[lc // 2][:, lc % 2,
                                          sc * 512:(sc + 1) * 512],
                        start=(lc == 0),
                        stop=(lc == 7),
                    )
            o_sb = osb_p.tile([128, S // 2], F32, name="o_sb", tag="osb")
            nc.vector.tensor_copy(o_sb, po)
            nc.sync.dma_start(
                outT[:, b * S + half * 1024:b * S + (half + 1) * 1024],
                o_sb)

        # ---- flat (batch, kc) software pipeline ----
        # ScalarE's exp stream is the bottleneck (~4.75us per kc unit); the
        # 64 units run back to back with NO break at batch boundaries, and
        # the PE is in-order, so every hook is placed where its DMA inputs
        # are guaranteed ready (a stalled instruction head-of-line blocks
        # everything behind it).  Per-unit PE load stays in [1.05, 5.3]us:
        #   even units >= 6: AV burst for units j-6/j-5 (pairs are batch-
        #     aligned since 16 is even); the 8th pair of a batch lands at
        #     kc4 of the next batch, followed directly by its z flushes
        #   kc5:  x load for b+1 (x8 on SWDGE, xT on HWDGE in parallel;
        #         issued after batch 0's V projection has consumed x(0))
        #   kc5/6:  z^T slab loads for b-1 (HWDGE; waits on its AllGather
        #           via the zfull dep, stalling only the DMA, not the PE)
        #   kc7/9:  out-projection halves for b-1
        #   kc10/12/14: V projection halves + V transpose for b+1
        #   kc11/13:    Q/K fp8 projections for b+1 (x8 landed by ~kc6)
        # Batch 0's projections (including V) run in the prelude: the vs
        # scaling inside scores_exp needs a program-order RAW dep on the
        # V transpose, so v_sb(0) must be written before unit 0 issues.
        x_cs = {0: load_x(0)}
        cur_q = proj_w8(wq_sb, "qt", x_cs[0])
        cur_k = proj_w8(wk_sb, "kt", x_cs[0])
        vt0 = vtpool.tile([128, S], BF16, name="vt", tag="vt")
        proj_w_half(wv_sb, vt0, x_cs[0], 0)
        proj_w_half(wv_sb, vt0, x_cs[0], 1)
        vsbs = {0: vpool.tile([128, KC, 128], BF16, name="v_sb", tag="v")}
        transpose_v(vt0, vsbs[0])
        vts = {}
        nxt = {}
        zacs = {}
        zfs = {}
        pending = []
        npairs = 0

        def av_step():
            nonlocal pending, npairs
            (b1, k1, p1), (b2, k2, p2) = pending[:2]
            pending = pending[2:]
            av_pair([(k1, p1), (k2, p2)], zacs[b1], first=(npairs % 8 == 0))
            npairs += 1
            return b1

        for j in range(B * KC):
            b, kc = divmod(j, KC)
            if kc == 0:
                zacs[b] = ztp.tile([128, S], F32, name="zac", tag="zac")
            pending.append((b, kc, scores_exp(kc, cur_q, cur_k, vsbs[b])))
            if j >= 6 and j % 2 == 0:
                fin = av_step()
                # 8 pairs complete a batch: its z flushes ride out here
                if npairs % 8 == 0:
                    flush_half(fin, zacs[fin], 0)
                    flush_half(fin, zacs[fin], 1)
            if b + 1 < B:
                if kc == 5:
                    x_cs[b + 1] = load_x(b + 1)
                elif kc == 10:
                    vts[b + 1] = vtpool.tile([128, S], BF16, name="vt",
                                             tag="vt")
                    proj_w_half(wv_sb, vts[b + 1], x_cs[b + 1], 0)
                elif kc == 11:
                    nxt["qt"] = proj_w8(wq_sb, "qt", x_cs[b + 1])
                elif kc == 12:
                    proj_w_half(wv_sb, vts[b + 1], x_cs[b + 1], 1)
                elif kc == 13:
                    nxt["kt"] = proj_w8(wk_sb, "kt", x_cs[b + 1])
                elif kc == 14:
                    vsbs[b + 1] = vpool.tile([128, KC, 128], BF16,
                                             name="v_sb", tag="v")
                    transpose_v(vts.pop(b + 1), vsbs[b + 1])
            if b >= 1:
                if kc == 5:
                    zfs[0] = outproj_load_half(b - 1, 0)
                elif kc == 6:
                    zfs[1] = outproj_load_half(b - 1, 1)
                elif kc == 7:
                    outproj_half(b - 1, 0, zfs[0])
                elif kc == 9:
                    outproj_half(b - 1, 1, zfs[1])
            if kc == 15 and b + 1 < B:
                cur_q, cur_k = nxt["qt"], nxt["kt"]
                nxt = {}
        # drain: last three AV bursts, then the final batch's flush +
        # out-projection (per-half, so AllGather latency overlaps the
        # first half's projection)
        while pending:
            av_step()
        flush_half(B - 1, zacs[B - 1], 0)
        zf0 = outproj_load_half(B - 1, 0)
        flush_half(B - 1, zacs[B - 1], 1)
        outproj_half(B - 1, 0, zf0)
        outproj_half(B - 1, 1, outproj_load_half(B - 1, 1))


def _legalize_waits(nc):
    """This walrus build accepts only ~2 sync commands (1 wait + 1 inc) per
    instruction for the standard engine/DMA templates; Tile can emit 2-3
    waits (WAR + WAW + RAW). Hoist all but one wait of any multi-wait
    instruction onto single-wait NOPs on the same engine, immediately
    before it — the raw-bass `wait_ge; op` pattern. Drain/EventSemaphore
    templates accept many waits (the kernel-tail barrier relies on it)."""
    import bass_rust

    n = 0
    for f in nc.m.functions:
        for blk in f.blocks:
            out = []
            changed = False
            for inst in blk.instructions:
                si = inst.sync_info
                if si is not None and len(si.on_wait) > 1:
                    for w in si.on_wait[:-1]:
                        n += 1
                        out.append(
                            bass_rust.InstNoOp(
                                name=f"I-hoistwait-{n}",
                                engine=inst.engine,
                                bass_nofuse=True,
                                sync_info=bass_rust.SyncInfo(
                                    on_wait=[w], on_update=[]
                                ),
                            )
                        )
                    inst.sync_info = bass_rust.SyncInfo(
                        on_wait=[si.on_wait[-1]], on_update=list(si.on_update)
                    )
                    changed = True
                out.append(inst)
            if changed:
                blk.instructions = out


def build(legalize=True):
    nc = bass.Bass(
        "TRN2",
        target_bir_lowering=False,
        debug=False,
        enable_asserts=False,
        num_devices=NCORES,
    )
    xT = nc.dram_tensor("xT", [B, D, S], BF16, kind="ExternalInput").ap()
    x8 = nc.dram_tensor("x8", [B, 4, 128, 2, S], F8, kind="ExternalInput").ap()
    wq8 = nc.dram_tensor("wq8", [128, 4, 2, LPC], F8, kind="ExternalInput").ap()
    wk8 = nc.dram_tensor("wk8", [128, 4, 2, LPC], F8, kind="ExternalInput").ap()
    wvT = nc.dram_tensor("wvT", [D, LPC], BF16, kind="ExternalInput").ap()
    woT = nc.dram_tensor("woT", [L, DPC], BF16, kind="ExternalInput").ap()
    outT = nc.dram_tensor("outT", [DPC, B * S], F32, kind="ExternalOutput").ap()

    with tile.TileContext(nc) as tc:
        from contextlib import ExitStack

        with ExitStack() as ctx:
            dram = ctx.enter_context(tc.tile_pool(name="dram", bufs=1, space="DRAM"))
            zloc = dram.tile([B, 2, LPC, S // 2], BF16, name="zloc")
            zfull = [
                dram.tile([L, S // 2], BF16, name=f"zfull{i}", tag=f"zfull{i}",
                          addr_space="Shared")
                for i in range(2 * B)
            ]
            _body(tc, xT, x8, wq8, wk8, wvT, woT, outT, zloc, zfull)
    if legalize:
        # the inserted NOPs are invisible to the simulator's race-detector
        # registry; sim callers pass legalize=False (identical semantics)
        _legalize_waits(nc)
    return nc


def make_in_maps(x, Wq, Wk, Wv, Wo):
    bf = ml_dtypes.bfloat16
    f8 = ml_dtypes.float8_e4m3
    x = np.asarray(x, np.float32)
    xTf = np.ascontiguousarray(x.transpose(0, 2, 1))            # (B, D, S)
    xT = xTf.astype(bf)
    # fp8 copy with D-chunk pairs interleaved for DoubleRow matmuls
    x8 = np.ascontiguousarray(
        xTf.reshape(B, 4, 2, 128, S).transpose(0, 1, 3, 2, 4)).astype(f8)
    WoT = np.ascontiguousarray(np.asarray(Wo, np.float32).T)    # (L, D)

    def w8(W, rs):
        wT = np.asarray(W, np.float32)[rs].T                    # (D, 128)
        return np.ascontiguousarray(
            wT.reshape(4, 2, 128, LPC).transpose(2, 0, 1, 3)).astype(f8)

    in_maps = []
    for c in range(NCORES):
        rs = slice(128 * c, 128 * (c + 1))
        in_maps.append({
            "xT": xT,
            "x8": x8,
            "wq8": w8(Wq, rs),
            "wk8": w8(Wk, rs),
            "wvT": np.ascontiguousarray(np.asarray(Wv, np.float32)[rs].T).astype(bf),
            "woT": np.ascontiguousarray(WoT[:, rs]).astype(bf),
        })
    return in_maps


def _install_ntff_hook_shim():
    """This container's `antenv` lacks `axon_hooks`; recreate the NTFF
    profile hook (same ctypes recipe as trn_agent_boot.trn_boot) so
    run_bass_kernel_spmd(trace=True) can capture exec_time_ns."""
    import sys
    import types
    import ctypes
    import contextlib

    try:
        import antenv.axon_hooks  # noqa: F401
        return
    except ImportError:
        pass

    hook = None
    so_path = os.environ.get("PJRT_LIBRARY_PATH")
    if so_path and os.path.exists(so_path):
        try:
            lib = ctypes.CDLL(so_path)
            if hasattr(lib, "axon_start_nrt_profile"):
                lib.axon_start_nrt_profile.argtypes = [
                    ctypes.POINTER(ctypes.c_int64),
                    ctypes.c_size_t,
                ]
                lib.axon_start_nrt_profile.restype = ctypes.c_int64
                lib.axon_stop_nrt_profile.argtypes = [ctypes.c_char_p]
                lib.axon_stop_nrt_profile.restype = ctypes.c_int64

                @contextlib.contextmanager
                def _hook(output_dir, device_ids):
                    import jax

                    jax.devices()
                    if device_ids:
                        ids = (ctypes.c_int64 * len(device_ids))(*device_ids)
                        rc = lib.axon_start_nrt_profile(ids, len(device_ids))
                    else:
                        rc = lib.axon_start_nrt_profile(None, 0)
                    if rc != 0:
                        raise RuntimeError(f"axon_start_nrt_profile rc={rc}")
                    try:
                        yield
                    finally:
                        n = lib.axon_stop_nrt_profile(str(output_dir).encode())
                        print(f"profile: {n} file(s) written to {output_dir}")

                hook = _hook
        except OSError:
            hook = None

    mod = types.ModuleType("antenv.axon_hooks")
    mod.get_axon_ntff_profile_hook = lambda: hook
    mod.set_axon_ntff_profile_hook = lambda h: None
    sys.modules["antenv.axon_hooks"] = mod
    import antenv

    antenv.axon_hooks = mod


def _gather(res):
    return np.concatenate(
        [np.asarray(res.results[c]["outT"], np.float32) for c in range(NCORES)],
        axis=0,
    )  # (D, B*S)


def kernel(x, Wq, Wk, Wv, Wo):
    global LAST_EXEC_NS
    in_maps = make_in_maps(x, Wq, Wk, Wv, Wo)
    nc = build()
    trace = bool(int(os.environ.get("BASS_KERNEL_TRACE", "0")))
    if trace:
        _install_ntff_hook_shim()
    core_ids = list(range(NCORES))
    # Run twice and cross-check: the first execution of a freshly-loaded
    # NEFF was once observed to produce a corrupted result; a re-run is
    # ~0.6ms of device time against a multi-second compile+load.
    r1 = _gather(run_bass_kernel_spmd(nc, in_maps, core_ids=core_ids))
    res = run_bass_kernel_spmd(nc, in_maps, core_ids=core_ids, trace=trace)
    LAST_EXEC_NS = res.exec_time_ns
    r2 = _gather(res)
    if not np.array_equal(r1, r2):
        r3 = _gather(run_bass_kernel_spmd(nc, in_maps, core_ids=core_ids))
        outT = r3 if np.array_equal(r2, r3) else (
            r1 if np.array_equal(r1, r3) else r2)
    else:
        outT = r2
    return np.ascontiguousarray(outT.T).reshape(B, S, D).astype(np.float32)

